# revision 20
# baseline (speedup 1.0000x reference)
"""Decoupled Contrastive Loss on 8 Trainium2 NeuronCores.

Strategy (data-parallel over row slabs, identical SPMD program, per-core
np.roll so every core sees its own slab at rows 0:1024):

Device (per core):
  - normalize both feature matrices (host-computed 1/norm, device row-scale)
    and cast to fp16; DMA-transpose into [D, B] layout in SBUF.
  - cross-modal pass: slab x full sim = vn @ tn^T, fused exp (+1/T scale)
    with per-row accumulation on ACT, column accumulation on DVE.
  - intra-modal passes (v@v^T, t@t^T) exploit symmetry: only quad-distance
    d in {1..8} tiles plus a triangular-masked diagonal tile are computed;
    row partials + column partials are combined on the host.
Host:
  - input prep: 1/norms, per-row match counts, np.roll per core, mask
    constants; mask-weighted raw-sim sums via group-sum identities
    (sum_match sim = (1/T) sum_g <Vg, Tg>).
  - combine per-core partials (f64), assemble the scalar loss.
"""

import numpy as np

TEMPERATURE = 0.07
LAMBDA_V = 0.5
LAMBDA_T = 0.5
B, D = 8192, 512
NC_CORES = 8
SLAB = B // NC_CORES      # 1024
MB = 128                  # out-tile partition dim
NB = 512                  # out-tile free dim
NM = SLAB // MB           # 8 m-blocks (slab rows)
NN = B // NB              # 16 n-blocks
KC = D // 128             # 4 contraction chunks
NCHUNK = B // MB          # 64 row chunks per matrix
VNB = 10                  # vT column blocks actually read (triangle)
VCHUNK = VNB * NB // MB   # 40 v row chunks needed
INV_T = 1.0 / TEMPERATURE

_BUILT = None


def _build():
    """Build the SPMD Bass program (once per process)."""
    import concourse.bacc as bacc
    import concourse.tile as tile
    from concourse import mybir

    f32 = mybir.dt.float32
    f16 = mybir.dt.float16
    bf16 = mybir.dt.bfloat16
    Exp = mybir.ActivationFunctionType.Exp
    mult = mybir.AluOpType.mult
    add = mybir.AluOpType.add
    AxX = mybir.AxisListType.X

    nc = bacc.Bacc(
        "TRN2", target_bir_lowering=False, debug=False,
        num_devices=NC_CORES)

    v_in = nc.dram_tensor("v", [B, D], f32, kind="ExternalInput")
    t_in = nc.dram_tensor("t", [B, D], f32, kind="ExternalInput")
    rnv_in = nc.dram_tensor("rnv", [MB, NCHUNK], f32, kind="ExternalInput")
    rnt_in = nc.dram_tensor("rnt", [MB, NCHUNK], f32, kind="ExternalInput")
    eye16_in = nc.dram_tensor("eye16", [MB, MB], f16, kind="ExternalInput")
    trimask_in = nc.dram_tensor("trimask", [MB, 4 * NB], bf16, kind="ExternalInput")

    rp_sim_out = nc.dram_tensor("rp_sim", [MB, NM, 8], f32, kind="ExternalOutput")
    ca_sim_out = nc.dram_tensor("ca_sim", [8, MB, 2 * NB], bf16, kind="ExternalOutput")
    rp_v_out = nc.dram_tensor("rp_v", [MB, NM, 5], f32, kind="ExternalOutput")
    rp_t_out = nc.dram_tensor("rp_t", [MB, NM, 5], f32, kind="ExternalOutput")
    ca_v_out = nc.dram_tensor("ca_v", [MB, 9 * NB], bf16, kind="ExternalOutput")
    ca_t_out = nc.dram_tensor("ca_t", [MB, 9 * NB], bf16, kind="ExternalOutput")

    with tile.TileContext(nc) as tc:
        from contextlib import ExitStack
        with ExitStack() as ctx:
            singles = ctx.enter_context(tc.tile_pool(name="singles", bufs=1))
            ldp = ctx.enter_context(tc.tile_pool(name="ldp", bufs=4))
            nhp = ctx.enter_context(tc.tile_pool(name="nhp", bufs=4))
            expp = ctx.enter_context(tc.tile_pool(name="expp", bufs=6))
            colp = ctx.enter_context(tc.tile_pool(name="colp", bufs=3))
            scrp = ctx.enter_context(tc.tile_pool(name="scrp", bufs=2))
            psum = ctx.enter_context(
                tc.tile_pool(name="psum", bufs=3, space="PSUM"))
            trp = ctx.enter_context(
                tc.tile_pool(name="trp", bufs=2, space="PSUM"))

            # ---- constants ----
            rn_sb = {}
            for name, src in (("v", rnv_in), ("t", rnt_in)):
                r = singles.tile([MB, NCHUNK], f32, tag=f"rn_{name}",
                                 name=f"rn_{name}")
                nc.sync.dma_start(out=r[:], in_=src[:])
                rn_sb[name] = r
            eye16_sb = singles.tile([MB, MB], f16, tag="eye16")
            nc.sync.dma_start(out=eye16_sb[:], in_=eye16_in[:])
            tri_sb = singles.tile([MB, 4 * NB], bf16, tag="tri")
            nc.sync.dma_start(out=tri_sb[:], in_=trimask_in[:])

            # pre-consume constants on DVE so downstream ops (walrus allows
            # only one sync-wait on TensorScalar etc.) don't need a second
            # wait on the constants' DMA queues.
            warm = singles.tile([MB, 1], f32, tag="warm", name="warm")
            for const_ap in (rn_sb["v"], rn_sb["t"], tri_sb):
                nc.vector.tensor_copy(warm[:], const_ap[:, 0:1])

            # ---- persistent transposed matrices ----
            xT = {"v": [], "t": []}
            for name, ncols in (("v", VNB * NB), ("t", B)):
                for k in range(KC):
                    xT[name].append(
                        singles.tile([MB, ncols], f16, tag=f"{name}T{k}",
                                     name=f"{name}T{k}"))

            def norm_transpose_chunks(name, src, chunks):
                """Per 128-row chunk: DMA load f32, DVE normalize->fp16,
                then transpose into the [D, B] layout: PE identity-matmul
                for v, DMA-XBAR (sbuf->sbuf) for t to keep PE lean."""
                for k in chunks:
                    ld = ldp.tile([MB, D], f32, tag="ld")
                    nc.sync.dma_start(
                        out=ld[:], in_=src[k * MB:(k + 1) * MB, :])
                    nh = nhp.tile([MB, D], f16, tag="nh")
                    nc.vector.tensor_scalar_mul(
                        nh[:], ld[:], rn_sb[name][:, k:k + 1])
                    for kc in range(KC):
                        if name == "t":
                            nc.sync.dma_start(
                                out=xT[name][kc][:, k * MB:(k + 1) * MB],
                                in_=nh[:, kc * MB:(kc + 1) * MB],
                                transpose=True)
                        else:
                            tp = trp.tile([MB, MB], f16, tag="tr")
                            nc.tensor.transpose(
                                tp[:], nh[:, kc * MB:(kc + 1) * MB],
                                eye16_sb[:])
                            nc.vector.tensor_copy(
                                xT[name][kc][:, k * MB:(k + 1) * MB], tp[:])

            # v slab first so the cross-modal pass can start early
            norm_transpose_chunks("v", v_in, range(NM))
            norm_transpose_chunks("t", t_in, range(NCHUNK))
            norm_transpose_chunks("v", v_in, range(NM, VCHUNK))

            def mm_half(ps, lhs_name, rhs_name, m, n, half):
                for k in range(KC):
                    nc.tensor.matmul(
                        ps[:, half * NB:(half + 1) * NB],
                        lhsT=xT[lhs_name][k][:, m * MB:(m + 1) * MB],
                        rhs=xT[rhs_name][k][:, n * NB:(n + 1) * NB],
                        start=(k == 0), stop=(k == KC - 1))

            # ---- cross-modal pass (double-wide psum: 2 n-blocks per exp) ----
            rp_sim = singles.tile([MB, NM, 8], f32, tag="rp_sim")
            for p in range(8):
                colacc = colp.tile([MB, 2 * NB], bf16, tag="col")
                for m in range(NM):
                    ps = psum.tile([MB, 2 * NB], f32, tag="mm")
                    mm_half(ps, "v", "t", m, 2 * p, 0)
                    mm_half(ps, "v", "t", m, 2 * p + 1, 1)
                    et = expp.tile([MB, 2 * NB], bf16, tag="exp")
                    nc.scalar.activation(
                        et[:], ps[:], Exp, scale=INV_T,
                        accum_out=rp_sim[:, m, p:p + 1])
                    if m == 0:
                        nc.vector.tensor_copy(colacc[:], et[:])
                    else:
                        nc.vector.tensor_add(colacc[:], colacc[:], et[:])
                nc.sync.dma_start(out=ca_sim_out[p], in_=colacc[:])
            nc.sync.dma_start(out=rp_sim_out[:], in_=rp_sim[:])

            # ---- intra-modal passes (symmetric triangle, paired tiles) ----
            for name, rp_out, ca_out in (
                    ("v", rp_v_out, ca_v_out),
                    ("t", rp_t_out, ca_t_out)):
                rp = singles.tile([MB, NM, 5], f32, tag=f"rp_{name}",
                                  name=f"rp_{name}")
                colb = singles.tile([MB, 9 * NB], bf16, tag=f"colb_{name}",
                                    name=f"colb_{name}")
                nc.vector.memset(colb[:], 0.0)
                for m in range(NM):
                    G = m // 4
                    # diagonal tile: strict triangular mask (diag excluded);
                    # masked row-sum covers j>i, colacc covers j<i
                    ps = psum.tile([MB, 2 * NB], f32, tag="mm")
                    mm_half(ps, name, name, m, G, 0)
                    et = expp.tile([MB, NB], bf16, tag="exp5")
                    nc.scalar.activation(
                        et[:], ps[:, 0:NB], Exp, scale=INV_T)
                    em = expp.tile([MB, NB], bf16, tag="em")
                    nc.vector.tensor_mul(
                        em[:], et[:],
                        tri_sb[:, (m % 4) * NB:(m % 4 + 1) * NB])
                    nc.vector.tensor_reduce(
                        rp[:, m, 0:1], em[:], axis=AxX, op=add)
                    nc.vector.tensor_add(
                        colb[:, G * NB:(G + 1) * NB],
                        colb[:, G * NB:(G + 1) * NB], em[:])
                    # pairs (d0, d0+1); d=8 half contributes row-side only
                    for i, d0 in enumerate((1, 3, 5, 7)):
                        n0 = G + d0
                        ps = psum.tile([MB, 2 * NB], f32, tag="mm")
                        mm_half(ps, name, name, m, n0, 0)
                        mm_half(ps, name, name, m, n0 + 1, 1)
                        et = expp.tile([MB, 2 * NB], bf16, tag="exp")
                        nc.scalar.activation(
                            et[:], ps[:], Exp, scale=INV_T,
                            accum_out=rp[:, m, 1 + i:2 + i])
                        for half in (0, 1):
                            if d0 + half <= 7:
                                n = n0 + half
                                nc.vector.tensor_add(
                                    colb[:, n * NB:(n + 1) * NB],
                                    colb[:, n * NB:(n + 1) * NB],
                                    et[:, half * NB:(half + 1) * NB])
                nc.sync.dma_start(out=ca_out[:], in_=colb[:])
                nc.sync.dma_start(out=rp_out[:], in_=rp[:])

    nc.compile()
    return nc


def _get_nc():
    global _BUILT
    if _BUILT is None:
        _BUILT = _build()
    return _BUILT


def _host_prep(v, t, ids):
    v64, t64 = v.astype(np.float64), t.astype(np.float64)
    rnv = (1.0 / np.sqrt((v64 * v64).sum(1))).astype(np.float32)
    rnt = (1.0 / np.sqrt((t64 * t64).sum(1))).astype(np.float32)
    vn = (v * rnv[:, None]).astype(np.float32)
    tn = (t * rnt[:, None]).astype(np.float32)

    cnt = np.bincount(ids, minlength=2048)[ids].astype(np.float64)
    npos = max(int((cnt - 1).sum()), 1)

    order = np.argsort(ids, kind="stable")
    ids_s = ids[order]
    starts = np.r_[0, 1 + np.flatnonzero(np.diff(ids_s))]
    Vg = np.add.reduceat(vn[order].astype(np.float64), starts, axis=0)
    Tg = np.add.reduceat(tn[order].astype(np.float64), starts, axis=0)
    return dict(
        rnv=rnv, rnt=rnt, vn=vn, tn=tn, cnt=cnt, npos=npos,
        sig_vt=(Vg * Tg).sum(), sig_vv=(Vg * Vg).sum(), sig_tt=(Tg * Tg).sum(),
        diag_vv=(vn.astype(np.float64) ** 2).sum(),
        diag_tt=(tn.astype(np.float64) ** 2).sum())


def _trimask():
    import ml_dtypes
    m = np.zeros((MB, 4 * NB), dtype=ml_dtypes.bfloat16)
    cols = np.arange(NB)[None, :]
    rows = np.arange(MB)[:, None]
    for a in range(4):
        m[:, a * NB:(a + 1) * NB] = (cols - 128 * a) > rows
    return m


def run(v, t, ids, trace=False):
    """Run device + host combine. Returns (loss, BassKernelResults)."""
    from concourse.bass_utils import run_bass_kernel_spmd

    v = np.ascontiguousarray(np.asarray(v, dtype=np.float32))
    t = np.ascontiguousarray(np.asarray(t, dtype=np.float32))
    ids = np.asarray(ids).astype(np.int64)

    prep = _host_prep(v, t, ids)
    eye16 = np.eye(MB, dtype=np.float16)
    tri = _trimask()

    in_maps = []
    for c in range(NC_CORES):
        s = SLAB * c
        in_maps.append({
            "v": np.roll(v, -s, axis=0),
            "t": np.roll(t, -s, axis=0),
            "rnv": np.ascontiguousarray(
                np.roll(prep["rnv"], -s).reshape(NCHUNK, MB).T),
            "rnt": np.ascontiguousarray(
                np.roll(prep["rnt"], -s).reshape(NCHUNK, MB).T),
            "eye16": eye16,
            "trimask": tri,
        })

    nc = _get_nc()
    res = run_bass_kernel_spmd(
        nc, in_maps, core_ids=list(range(NC_CORES)), trace=trace)

    loss = _combine(res.results, prep)
    return loss, res


def _combine(results, prep):
    cnt, npos = prep["cnt"], prep["npos"]
    rowsum_sim = np.zeros(B)
    S_col = np.zeros(B)
    acc = {name: dict(row=np.zeros(B), col=np.zeros(B))
           for name in ("v", "t")}
    for c in range(NC_CORES):
        r = results[c]
        s = SLAB * c
        gsl = slice(s, s + SLAB)
        # rowpart_sim [128, 8, 8] -> local row 128*m+p = sum over npair
        rps = r["rp_sim"].astype(np.float64)
        rowsum_sim[gsl] += rps.sum(axis=2).T.reshape(SLAB)
        # colacc_sim [8, 128, 1024] -> local col 1024*p+f = sum over partitions
        cas = r["ca_sim"].astype(np.float64)
        S_col += np.roll(cas.sum(axis=1).reshape(B), s)
        for name in ("v", "t"):
            rp = r[f"rp_{name}"].astype(np.float64)       # [128, 8, 5]
            acc[name]["row"][gsl] += rp.sum(axis=2).T.reshape(SLAB)
            ca = r[f"ca_{name}"].astype(np.float64)       # [128, 9*512]
            colfull = np.zeros(B)
            colfull[:9 * NB] = ca.sum(axis=0)
            acc[name]["col"] += np.roll(colfull, s)

    lse_row = np.log(rowsum_sim)
    lse_col = np.log(S_col)
    v2t = (cnt @ lse_row - prep["sig_vt"] * INV_T) / npos
    t2v = (cnt @ lse_col - prep["sig_vt"] * INV_T) / npos

    inst = {}
    for name, sig, diag_raw in (("v", prep["sig_vv"], prep["diag_vv"]),
                                ("t", prep["sig_tt"], prep["diag_tt"])):
        a = acc[name]
        rs = a["row"] + a["col"]
        lse = np.log(rs)
        inst[name] = ((cnt - 1) @ lse - (sig - diag_raw) * INV_T) / npos

    total = 0.5 * (v2t + t2v) + LAMBDA_V * inst["v"] + LAMBDA_T * inst["t"]
    return np.float32(total)


def kernel(vision_features, text_features, match_ids):
    loss, _ = run(vision_features, text_features, match_ids)
    return np.array(loss, dtype=np.float32)


# revision 25
# speedup vs baseline: 2.3305x; 2.3305x over previous
"""Decoupled Contrastive Loss on 8 Trainium2 NeuronCores.

Strategy (data-parallel over row slabs, identical SPMD program, per-core
np.roll so every core sees its own slab at rows 0:1024):

Device (per core):
  - normalize both feature matrices (host-computed 1/norm, device row-scale)
    and cast to fp16; DMA-transpose into [D, B] layout in SBUF.
  - cross-modal pass: slab x full sim = vn @ tn^T, fused exp (+1/T scale)
    with per-row accumulation on ACT, column accumulation on DVE.
  - intra-modal passes (v@v^T, t@t^T) exploit symmetry: only quad-distance
    d in {1..8} tiles plus a triangular-masked diagonal tile are computed;
    row partials + column partials are combined on the host.
Host:
  - input prep: 1/norms, per-row match counts, np.roll per core, mask
    constants; mask-weighted raw-sim sums via group-sum identities
    (sum_match sim = (1/T) sum_g <Vg, Tg>).
  - combine per-core partials (f64), assemble the scalar loss.
"""

import numpy as np

TEMPERATURE = 0.07
LAMBDA_V = 0.5
LAMBDA_T = 0.5
B, D = 8192, 512
NC_CORES = 8
SLAB = B // NC_CORES      # 1024
MB = 128                  # out-tile partition dim
NB = 512                  # out-tile free dim
NM = SLAB // MB           # 8 m-blocks (slab rows)
NN = B // NB              # 16 n-blocks
KC = D // 128             # 4 contraction chunks
NCHUNK = B // MB          # 64 row chunks per matrix
VNB = 10                  # vT column blocks actually read (triangle)
VCHUNK = VNB * NB // MB   # 40 v row chunks needed
INV_T = 1.0 / TEMPERATURE
FP8_SCALE = 16.0          # features scaled into e4m3 range; dots carry 256x

_BUILT = None


def _build():
    """Build the SPMD Bass program (once per process)."""
    import concourse.bacc as bacc
    import concourse.tile as tile
    from concourse import mybir

    f32 = mybir.dt.float32
    f16 = mybir.dt.float16
    bf16 = mybir.dt.bfloat16
    f8 = mybir.dt.float8e4
    DR = mybir.MatmulPerfMode.DoubleRow
    INV_TS = INV_T / (FP8_SCALE * FP8_SCALE)
    Exp = mybir.ActivationFunctionType.Exp
    mult = mybir.AluOpType.mult
    add = mybir.AluOpType.add
    AxX = mybir.AxisListType.X

    nc = bacc.Bacc(
        "TRN2", target_bir_lowering=False, debug=False,
        num_devices=NC_CORES)

    v_in = nc.dram_tensor("v", [B, D], f32, kind="ExternalInput")
    t_in = nc.dram_tensor("t", [B, D], f32, kind="ExternalInput")
    rnv_in = nc.dram_tensor("rnv", [MB, NCHUNK], f32, kind="ExternalInput")
    rnt_in = nc.dram_tensor("rnt", [MB, NCHUNK], f32, kind="ExternalInput")
    eye16_in = nc.dram_tensor("eye16", [MB, MB], f16, kind="ExternalInput")
    trimask_in = nc.dram_tensor("trimask", [MB, 4 * NB], bf16, kind="ExternalInput")

    rp_sim_out = nc.dram_tensor("rp_sim", [MB, NM, 8], f32, kind="ExternalOutput")
    ca_sim_out = nc.dram_tensor("ca_sim", [8, MB, 2 * NB], bf16, kind="ExternalOutput")
    rp_v_out = nc.dram_tensor("rp_v", [MB, NM, 5], f32, kind="ExternalOutput")
    rp_t_out = nc.dram_tensor("rp_t", [MB, NM, 5], f32, kind="ExternalOutput")
    ca_v_out = nc.dram_tensor("ca_v", [MB, 9 * NB], bf16, kind="ExternalOutput")
    ca_t_out = nc.dram_tensor("ca_t", [MB, 9 * NB], bf16, kind="ExternalOutput")

    with tile.TileContext(nc) as tc:
        from contextlib import ExitStack
        with ExitStack() as ctx:
            singles = ctx.enter_context(tc.tile_pool(name="singles", bufs=1))
            ldp = ctx.enter_context(tc.tile_pool(name="ldp", bufs=4))
            nhp = ctx.enter_context(tc.tile_pool(name="nhp", bufs=4))
            expp = ctx.enter_context(tc.tile_pool(name="expp", bufs=6))
            colp = ctx.enter_context(tc.tile_pool(name="colp", bufs=3))
            scrp = ctx.enter_context(tc.tile_pool(name="scrp", bufs=2))
            psum = ctx.enter_context(
                tc.tile_pool(name="psum", bufs=3, space="PSUM"))
            trp = ctx.enter_context(
                tc.tile_pool(name="trp", bufs=2, space="PSUM"))

            # ---- constants ----
            rn_sb = {}
            for name, src in (("v", rnv_in), ("t", rnt_in)):
                r = singles.tile([MB, NCHUNK], f32, tag=f"rn_{name}",
                                 name=f"rn_{name}")
                nc.sync.dma_start(out=r[:], in_=src[:])
                rn_sb[name] = r
            eye16_sb = singles.tile([MB, MB], f16, tag="eye16")
            nc.sync.dma_start(out=eye16_sb[:], in_=eye16_in[:])
            tri_sb = singles.tile([MB, 4 * NB], bf16, tag="tri")
            nc.sync.dma_start(out=tri_sb[:], in_=trimask_in[:])

            # pre-consume constants on DVE so downstream ops (walrus allows
            # only one sync-wait on TensorScalar etc.) don't need a second
            # wait on the constants' DMA queues.
            warm = singles.tile([MB, 1], f32, tag="warm", name="warm")
            for const_ap in (rn_sb["v"], rn_sb["t"], tri_sb):
                nc.vector.tensor_copy(warm[:], const_ap[:, 0:1])

            # ---- persistent transposed matrices ----
            xT = {"v": [], "t": []}
            xT3 = {"v": [], "t": []}
            NCOLS = {"v": VNB * NB, "t": B}
            for name in ("v", "t"):
                for kp in range(2):
                    tl = singles.tile([MB, 2 * NCOLS[name]], f8,
                                      tag=f"{name}T{kp}", name=f"{name}T{kp}")
                    xT[name].append(tl)
                    xT3[name].append(
                        tl.rearrange("p (two c) -> p two c", two=2))

            def norm_transpose_chunks(name, src, chunks):
                """Per 128-row chunk: DMA load f32, DVE normalize->fp16,
                then transpose into the [D, B] layout: PE identity-matmul
                for v, DMA-XBAR (sbuf->sbuf) for t to keep PE lean."""
                for k in chunks:
                    ld = ldp.tile([MB, D], f32, tag="ld")
                    nc.sync.dma_start(
                        out=ld[:], in_=src[k * MB:(k + 1) * MB, :])
                    nh = nhp.tile([MB, D], f16, tag="nh")
                    nc.vector.tensor_scalar_mul(
                        nh[:], ld[:], rn_sb[name][:, k:k + 1])
                    for kc in range(KC):
                        tp = trp.tile([MB, MB], f16, tag="tr")
                        nc.tensor.transpose(
                            tp[:], nh[:, kc * MB:(kc + 1) * MB],
                            eye16_sb[:])
                        nc.vector.tensor_copy(
                            xT[name][kc // 2][:, (kc % 2) * NCOLS[name]
                                              + k * MB:
                                              (kc % 2) * NCOLS[name]
                                              + (k + 1) * MB], tp[:])

            # v slab first so the cross-modal pass can start early
            norm_transpose_chunks("v", v_in, range(NM))
            norm_transpose_chunks("t", t_in, range(NCHUNK))
            norm_transpose_chunks("v", v_in, range(NM, VCHUNK))

            def mm_half(ps, lhs_name, rhs_name, m, n, half):
                for kp in range(2):
                    nc.tensor.matmul(
                        ps[:, half * NB:(half + 1) * NB],
                        lhsT=xT3[lhs_name][kp][:, :, m * MB:(m + 1) * MB],
                        rhs=xT3[rhs_name][kp][:, :, n * NB:(n + 1) * NB],
                        start=(kp == 0), stop=(kp == 1),
                        perf_mode=DR)

            # ---- cross-modal pass (double-wide psum: 2 n-blocks per exp) ----
            rp_sim = singles.tile([MB, NM, 8], f32, tag="rp_sim")
            for p in range(8):
                colacc = colp.tile([MB, 2 * NB], bf16, tag="col")
                for m in range(NM):
                    ps = psum.tile([MB, 2 * NB], f32, tag="mm")
                    mm_half(ps, "v", "t", m, 2 * p, 0)
                    mm_half(ps, "v", "t", m, 2 * p + 1, 1)
                    et = expp.tile([MB, 2 * NB], bf16, tag="exp")
                    nc.scalar.activation(
                        et[:], ps[:], Exp, scale=INV_TS,
                        accum_out=rp_sim[:, m, p:p + 1])
                    if m == 0:
                        nc.vector.tensor_copy(colacc[:], et[:])
                    else:
                        nc.vector.tensor_add(colacc[:], colacc[:], et[:])
                nc.sync.dma_start(out=ca_sim_out[p], in_=colacc[:])
            nc.sync.dma_start(out=rp_sim_out[:], in_=rp_sim[:])

            # ---- intra-modal passes (symmetric triangle, paired tiles) ----
            for name, rp_out, ca_out in (
                    ("v", rp_v_out, ca_v_out),
                    ("t", rp_t_out, ca_t_out)):
                rp = singles.tile([MB, NM, 5], f32, tag=f"rp_{name}",
                                  name=f"rp_{name}")
                colb = singles.tile([MB, 9 * NB], bf16, tag=f"colb_{name}",
                                    name=f"colb_{name}")
                nc.vector.memset(colb[:], 0.0)
                for m in range(NM):
                    G = m // 4
                    # diagonal tile: strict triangular mask (diag excluded);
                    # masked row-sum covers j>i, colacc covers j<i
                    ps = psum.tile([MB, 2 * NB], f32, tag="mm")
                    mm_half(ps, name, name, m, G, 0)
                    et = expp.tile([MB, NB], bf16, tag="exp5")
                    nc.scalar.activation(
                        et[:], ps[:, 0:NB], Exp, scale=INV_TS)
                    em = expp.tile([MB, NB], bf16, tag="em")
                    nc.vector.tensor_mul(
                        em[:], et[:],
                        tri_sb[:, (m % 4) * NB:(m % 4 + 1) * NB])
                    nc.vector.tensor_reduce(
                        rp[:, m, 0:1], em[:], axis=AxX, op=add)
                    nc.vector.tensor_add(
                        colb[:, G * NB:(G + 1) * NB],
                        colb[:, G * NB:(G + 1) * NB], em[:])
                    # pairs (d0, d0+1); d=8 half contributes row-side only
                    for i, d0 in enumerate((1, 3, 5, 7)):
                        n0 = G + d0
                        ps = psum.tile([MB, 2 * NB], f32, tag="mm")
                        mm_half(ps, name, name, m, n0, 0)
                        mm_half(ps, name, name, m, n0 + 1, 1)
                        et = expp.tile([MB, 2 * NB], bf16, tag="exp")
                        nc.scalar.activation(
                            et[:], ps[:], Exp, scale=INV_TS,
                            accum_out=rp[:, m, 1 + i:2 + i])
                        for half in (0, 1):
                            if d0 + half <= 7:
                                n = n0 + half
                                nc.vector.tensor_add(
                                    colb[:, n * NB:(n + 1) * NB],
                                    colb[:, n * NB:(n + 1) * NB],
                                    et[:, half * NB:(half + 1) * NB])
                nc.sync.dma_start(out=ca_out[:], in_=colb[:])
                nc.sync.dma_start(out=rp_out[:], in_=rp[:])

    nc.compile()
    return nc


def _get_nc():
    global _BUILT
    if _BUILT is None:
        _BUILT = _build()
    return _BUILT


def _host_prep(v, t, ids):
    v64, t64 = v.astype(np.float64), t.astype(np.float64)
    rnv = (1.0 / np.sqrt((v64 * v64).sum(1))).astype(np.float32)
    rnt = (1.0 / np.sqrt((t64 * t64).sum(1))).astype(np.float32)
    vn = (v * rnv[:, None]).astype(np.float32)
    tn = (t * rnt[:, None]).astype(np.float32)

    cnt = np.bincount(ids, minlength=2048)[ids].astype(np.float64)
    npos = max(int((cnt - 1).sum()), 1)

    order = np.argsort(ids, kind="stable")
    ids_s = ids[order]
    starts = np.r_[0, 1 + np.flatnonzero(np.diff(ids_s))]
    Vg = np.add.reduceat(vn[order].astype(np.float64), starts, axis=0)
    Tg = np.add.reduceat(tn[order].astype(np.float64), starts, axis=0)
    return dict(
        rnv=rnv, rnt=rnt, vn=vn, tn=tn, cnt=cnt, npos=npos,
        sig_vt=(Vg * Tg).sum(), sig_vv=(Vg * Vg).sum(), sig_tt=(Tg * Tg).sum(),
        diag_vv=(vn.astype(np.float64) ** 2).sum(),
        diag_tt=(tn.astype(np.float64) ** 2).sum())


def _trimask():
    import ml_dtypes
    m = np.zeros((MB, 4 * NB), dtype=ml_dtypes.bfloat16)
    cols = np.arange(NB)[None, :]
    rows = np.arange(MB)[:, None]
    for a in range(4):
        m[:, a * NB:(a + 1) * NB] = (cols - 128 * a) > rows
    return m


def run(v, t, ids, trace=False):
    """Run device + host combine. Returns (loss, BassKernelResults)."""
    from concourse.bass_utils import run_bass_kernel_spmd

    v = np.ascontiguousarray(np.asarray(v, dtype=np.float32))
    t = np.ascontiguousarray(np.asarray(t, dtype=np.float32))
    ids = np.asarray(ids).astype(np.int64)

    prep = _host_prep(v, t, ids)
    eye16 = np.eye(MB, dtype=np.float16)
    tri = _trimask()

    in_maps = []
    for c in range(NC_CORES):
        s = SLAB * c
        in_maps.append({
            "v": np.roll(v, -s, axis=0),
            "t": np.roll(t, -s, axis=0),
            "rnv": np.ascontiguousarray(
                np.roll(prep["rnv"] * FP8_SCALE, -s).reshape(NCHUNK, MB).T
                ).astype(np.float32),
            "rnt": np.ascontiguousarray(
                np.roll(prep["rnt"] * FP8_SCALE, -s).reshape(NCHUNK, MB).T
                ).astype(np.float32),
            "eye16": eye16,
            "trimask": tri,
        })

    nc = _get_nc()
    res = run_bass_kernel_spmd(
        nc, in_maps, core_ids=list(range(NC_CORES)), trace=trace)

    loss = _combine(res.results, prep)
    return loss, res


def _combine(results, prep):
    cnt, npos = prep["cnt"], prep["npos"]
    rowsum_sim = np.zeros(B)
    S_col = np.zeros(B)
    acc = {name: dict(row=np.zeros(B), col=np.zeros(B))
           for name in ("v", "t")}
    for c in range(NC_CORES):
        r = results[c]
        s = SLAB * c
        gsl = slice(s, s + SLAB)
        # rowpart_sim [128, 8, 8] -> local row 128*m+p = sum over npair
        rps = r["rp_sim"].astype(np.float64)
        rowsum_sim[gsl] += rps.sum(axis=2).T.reshape(SLAB)
        # colacc_sim [8, 128, 1024] -> local col 1024*p+f = sum over partitions
        cas = r["ca_sim"].astype(np.float64)
        S_col += np.roll(cas.sum(axis=1).reshape(B), s)
        for name in ("v", "t"):
            rp = r[f"rp_{name}"].astype(np.float64)       # [128, 8, 5]
            acc[name]["row"][gsl] += rp.sum(axis=2).T.reshape(SLAB)
            ca = r[f"ca_{name}"].astype(np.float64)       # [128, 9*512]
            colfull = np.zeros(B)
            colfull[:9 * NB] = ca.sum(axis=0)
            acc[name]["col"] += np.roll(colfull, s)

    lse_row = np.log(rowsum_sim)
    lse_col = np.log(S_col)
    v2t = (cnt @ lse_row - prep["sig_vt"] * INV_T) / npos
    t2v = (cnt @ lse_col - prep["sig_vt"] * INV_T) / npos

    inst = {}
    for name, sig, diag_raw in (("v", prep["sig_vv"], prep["diag_vv"]),
                                ("t", prep["sig_tt"], prep["diag_tt"])):
        a = acc[name]
        rs = a["row"] + a["col"]
        lse = np.log(rs)
        inst[name] = ((cnt - 1) @ lse - (sig - diag_raw) * INV_T) / npos

    total = 0.5 * (v2t + t2v) + LAMBDA_V * inst["v"] + LAMBDA_T * inst["t"]
    return np.float32(total)


def kernel(vision_features, text_features, match_ids):
    loss, _ = run(vision_features, text_features, match_ids)
    return np.array(loss, dtype=np.float32)
